# revision 3
# baseline (speedup 1.0000x reference)
"""Trainium2 Bass kernel for nn_ComplexProjMeasurement.

Math (reference): for batch j (B=128), output dim i (D=512):
  out[j,i] = kr_i^T R_j kr_i + ki_i^T R_j ki_i - ki_i^T I_j kr_i + kr_i^T I_j ki_i
where kr = kernel[:,:,0], ki = kernel[:,:,1] (rows kr_i = kr[i,:]),
R_j = input_real[j], I_j = input_imag[j].

Factorization used here:
  Re_j = kr @ R_j - ki @ I_j          [i, b]
  Im_j = ki @ R_j + kr @ I_j          [i, b]
  out[j, i] = sum_b Re_j[i,b]*kr[i,b] + Im_j[i,b]*ki[i,b]

On-device: the two PSUM accumulations (Re, Im) are built from 8 fp32r
matmuls each ([K=128,M=128,N=512] tiles, contracting a over 4 k-tiles,
with host-negated -ki^T providing the subtraction); the diagonal
contraction is two fused multiply+reduce (tensor_tensor_reduce) DVE ops.

Sharding: data-parallel over batch; each of the 8 cores handles 16 j's.
"""

import numpy as np

import concourse.bass as bass
import concourse.mybir as mybir
import concourse.tile as tile
from concourse import bacc
from concourse.bass_utils import run_bass_kernel_spmd

F32R = mybir.dt.float32r
F32 = mybir.dt.float32

B = 128          # full batch
D = 512          # embed dim
NCORES = 8
JPC = B // NCORES          # j's per core = 16
JG = 4                     # j-group size (PSUM: 2 banks per j -> 8 banks)
NGROUPS = JPC // JG        # 4 groups per core
KT = D // 128              # 4 k-tiles (contraction)
MT = D // 128              # 4 m-tiles (output i)

_cached_nc = None


def _build_nc():
    nc = bacc.Bacc(None, target_bir_lowering=False)

    r = nc.dram_tensor("r", [JPC, D, D], F32R, kind="ExternalInput")
    im = nc.dram_tensor("im", [JPC, D, D], F32R, kind="ExternalInput")
    krT = nc.dram_tensor("krT", [D, D], F32R, kind="ExternalInput")
    kiT = nc.dram_tensor("kiT", [D, D], F32R, kind="ExternalInput")
    nkiT = nc.dram_tensor("nkiT", [D, D], F32R, kind="ExternalInput")
    kr = nc.dram_tensor("kr", [D, D], F32, kind="ExternalInput")
    ki = nc.dram_tensor("ki", [D, D], F32, kind="ExternalInput")
    out = nc.dram_tensor("out", [JPC, D], F32, kind="ExternalOutput")

    MUL = mybir.AluOpType.mult
    ADD = mybir.AluOpType.add

    with tile.TileContext(nc) as tc:
        with (
            tc.tile_pool(name="singles", bufs=1) as singles,
            tc.tile_pool(name="rpool", bufs=2) as rpool,
            tc.tile_pool(name="ipool", bufs=2) as ipool,
            tc.tile_pool(name="scr", bufs=2) as scrp,
            tc.tile_pool(name="ps", bufs=1, space="PSUM") as psp,
        ):
            # --- one-time loads -------------------------------------------
            # weights, [a, i] transposed layouts tiled as [p, kt, i]
            krT_s = singles.tile([128, KT, D], F32R, tag="krT")
            kiT_s = singles.tile([128, KT, D], F32R, tag="kiT")
            nkiT_s = singles.tile([128, KT, D], F32R, tag="nkiT")
            for t, src in ((krT_s, krT), (kiT_s, kiT), (nkiT_s, nkiT)):
                nc.sync.dma_start(
                    out=t, in_=src.rearrange("(kt p) i -> p kt i", p=128)
                )
            # natural layouts for the diagonal contraction, [p, m, b]
            kr_s = singles.tile([128, MT, D], F32, tag="kr")
            ki_s = singles.tile([128, MT, D], F32, tag="ki")
            for t, src in ((kr_s, kr), (ki_s, ki)):
                nc.sync.dma_start(
                    out=t, in_=src.rearrange("(m p) b -> p m b", p=128)
                )

            out_buf = singles.tile([128, JPC, MT], F32, tag="out_buf")

            # --- main loop ------------------------------------------------
            for jg in range(NGROUPS):
                rt = []
                it = []
                for jj in range(JG):
                    j = jg * JG + jj
                    rt.append(rpool.tile([128, KT, D], F32R, tag=f"r{jj}", name=f"r{jg}_{jj}"))
                    it.append(ipool.tile([128, KT, D], F32R, tag=f"i{jj}", name=f"i{jg}_{jj}"))
                    for kt in range(KT):
                        nc.sync.dma_start(
                            out=rt[jj][:, kt, :],
                            in_=r[j, kt * 128:(kt + 1) * 128, :],
                        )
                        nc.sync.dma_start(
                            out=it[jj][:, kt, :],
                            in_=im[j, kt * 128:(kt + 1) * 128, :],
                        )

                for m in range(MT):
                    ms = bass.ts(m, 128)
                    ps_re = [psp.tile([128, D], F32, tag=f"re{jj}", name=f"re{jg}_{m}_{jj}")
                             for jj in range(JG)]
                    ps_im = [psp.tile([128, D], F32, tag=f"imm{jj}", name=f"im{jg}_{m}_{jj}")
                             for jj in range(JG)]
                    for kt in range(KT):
                        first = kt == 0
                        last = kt == KT - 1
                        # weight krT[kt, m]: Re += kr@R ; Im += kr@I
                        for jj in range(JG):
                            nc.tensor.matmul(
                                ps_re[jj][:, :], krT_s[:, kt, ms],
                                rt[jj][:, kt, :], start=first, stop=False,
                            )
                        for jj in range(JG):
                            nc.tensor.matmul(
                                ps_im[jj][:, :], krT_s[:, kt, ms],
                                it[jj][:, kt, :], start=first, stop=False,
                            )
                        # weight kiT[kt, m]: Im += ki@R
                        for jj in range(JG):
                            nc.tensor.matmul(
                                ps_im[jj][:, :], kiT_s[:, kt, ms],
                                rt[jj][:, kt, :], start=False, stop=last,
                            )
                        # weight -kiT[kt, m]: Re -= ki@I
                        for jj in range(JG):
                            nc.tensor.matmul(
                                ps_re[jj][:, :], nkiT_s[:, kt, ms],
                                it[jj][:, kt, :], start=False, stop=last,
                            )

                    # diagonal contraction: out[j, m-tile] =
                    #   rowsum(Re*kr_m) + rowsum(Im*ki_m)
                    for jj in range(JG):
                        j = jg * JG + jj
                        scr = scrp.tile([128, 2, D], F32, tag="scr",
                                        name=f"scr{jg}_{m}_{jj}")
                        nc.vector.tensor_tensor(
                            out=scr[:, 0, :], in0=ps_re[jj][:, :],
                            in1=kr_s[:, m, :], op=MUL)
                        nc.vector.tensor_tensor(
                            out=scr[:, 1, :], in0=ps_im[jj][:, :],
                            in1=ki_s[:, m, :], op=MUL)
                        nc.vector.tensor_reduce(
                            out=out_buf[:, j, m:m + 1], in_=scr[:, :, :],
                            axis=mybir.AxisListType.XY, op=ADD)

            # --- store ----------------------------------------------------
            nc.sync.dma_start(
                out=out.rearrange("j (m p) -> p j m", p=128),
                in_=out_buf[:, :, :],
            )

    nc.finalize()
    return nc


def _get_nc():
    global _cached_nc
    if _cached_nc is None:
        _cached_nc = _build_nc()
    return _cached_nc


def make_in_maps(input_real, input_imag, kernel):
    input_real = np.ascontiguousarray(input_real, dtype=np.float32)
    input_imag = np.ascontiguousarray(input_imag, dtype=np.float32)
    kernel = np.asarray(kernel, dtype=np.float32)
    kr = np.ascontiguousarray(kernel[:, :, 0])
    ki = np.ascontiguousarray(kernel[:, :, 1])
    krT = np.ascontiguousarray(kr.T)
    kiT = np.ascontiguousarray(ki.T)
    nkiT = np.ascontiguousarray(-kiT)
    in_maps = []
    for c in range(NCORES):
        sl = slice(c * JPC, (c + 1) * JPC)
        in_maps.append({
            "r": input_real[sl],
            "im": input_imag[sl],
            "krT": krT,
            "kiT": kiT,
            "nkiT": nkiT,
            "kr": kr,
            "ki": ki,
        })
    return in_maps


def kernel(input_real, input_imag, kernel):
    nc = _get_nc()
    in_maps = make_in_maps(input_real, input_imag, kernel)
    res = run_bass_kernel_spmd(nc, in_maps, core_ids=list(range(NCORES)))
    return np.concatenate(
        [res.results[c]["out"] for c in range(NCORES)], axis=0
    ).astype(np.float32)


# revision 6
# speedup vs baseline: 528.5270x; 528.5270x over previous
"""Trainium2 Bass kernel for nn_ComplexProjMeasurement.

Math (reference): for batch j (B=128), output dim i (D=512):
  out[j,i] = kr_i^T R_j kr_i + ki_i^T R_j ki_i - ki_i^T I_j kr_i + kr_i^T I_j ki_i
where kr = kernel[:,:,0], ki = kernel[:,:,1] (rows kr_i = kr[i,:]),
R_j = input_real[j], I_j = input_imag[j].

Karatsuba factorization (3 matmul streams instead of 4):
  m1 = kr @ R_j,  m2 = ki @ I_j,  m3 = (kr+ki) @ (R_j+I_j)
  Re = m1 - m2,   Im = m3 - m1 - m2
  out[j,i] = sum_b Re*kr + Im*ki
           = sum_b m1*(kr-ki) + m2*(-(kr+ki)) + m3*ki    (all [i, b])

On-device: per (j, m-tile) three PSUM banks accumulate m1/m2/m3 from 4
fp32r matmuls each ([K=128,M=128,N=512], contracting a over 4 k-tiles);
the diagonal contraction: ScalarE drains two banks to SBUF, GpSimd does
those two elementwise multiplies from SBUF (it has no PSUM port), VectorE
does the third multiply straight from PSUM plus the final reduce. R+I is
precomputed on host and shipped as a third input stream.

Sharding: data-parallel over batch; each of the 8 cores handles 16 j's.
"""

import contextlib

import numpy as np

import concourse.bass as bass
import concourse.mybir as mybir
import concourse.tile as tile
from concourse import bacc
from concourse.bass_utils import run_bass_kernel_spmd

F32R = mybir.dt.float32r
F32 = mybir.dt.float32

B = 128          # full batch
D = 512          # embed dim
NCORES = 8
JPC = B // NCORES          # j's per core = 16
JG = 2                     # j-group size (PSUM: 3 banks per j -> 6 banks)
NGROUPS = JPC // JG        # 8 groups per core
KT = D // 128              # 4 k-tiles (contraction)
MT = D // 128              # 4 m-tiles (output i)

_cached_nc = None


def _build_nc(repeat=1):
    nc = bacc.Bacc(None, target_bir_lowering=False)

    r = nc.dram_tensor("r", [JPC, D, D], F32R, kind="ExternalInput")
    im = nc.dram_tensor("im", [JPC, D, D], F32R, kind="ExternalInput")
    rpi = nc.dram_tensor("rpi", [JPC, D, D], F32R, kind="ExternalInput")
    krT = nc.dram_tensor("krT", [D, D], F32R, kind="ExternalInput")
    kiT = nc.dram_tensor("kiT", [D, D], F32R, kind="ExternalInput")
    kpkT = nc.dram_tensor("kpkT", [D, D], F32R, kind="ExternalInput")
    d1 = nc.dram_tensor("d1", [D, D], F32, kind="ExternalInput")
    d2 = nc.dram_tensor("d2", [D, D], F32, kind="ExternalInput")
    d3 = nc.dram_tensor("d3", [D, D], F32, kind="ExternalInput")
    out = nc.dram_tensor("out", [JPC, D], F32, kind="ExternalOutput")

    MUL = mybir.AluOpType.mult
    ADD = mybir.AluOpType.add

    with tile.TileContext(nc) as tc:
        with (
            tc.tile_pool(name="singles", bufs=1) as singles,
            tc.tile_pool(name="rpool", bufs=2) as rpool,
            tc.tile_pool(name="ipool", bufs=2) as ipool,
            tc.tile_pool(name="ppool", bufs=2) as ppool,
            tc.tile_pool(name="scr", bufs=3) as scrp,
            tc.tile_pool(name="ps", bufs=1, space="PSUM") as psp,
        ):
            # --- one-time loads -------------------------------------------
            # matmul weights, transposed [a, i] layouts tiled as [p, kt, i]
            krT_s = singles.tile([128, KT, D], F32R, tag="krT")
            kiT_s = singles.tile([128, KT, D], F32R, tag="kiT")
            kpkT_s = singles.tile([128, KT, D], F32R, tag="kpkT")
            for t, src in ((krT_s, krT), (kiT_s, kiT), (kpkT_s, kpkT)):
                nc.sync.dma_start(
                    out=t, in_=src.rearrange("(kt p) i -> p kt i", p=128)
                )
            # diag-contraction combos, natural [i, b] layout as [p, m, b]
            d1_s = singles.tile([128, MT, D], F32, tag="d1")
            d2_s = singles.tile([128, MT, D], F32, tag="d2")
            d3_s = singles.tile([128, MT, D], F32, tag="d3")
            for t, src in ((d1_s, d1), (d2_s, d2), (d3_s, d3)):
                nc.sync.dma_start(
                    out=t, in_=src.rearrange("(m p) b -> p m b", p=128)
                )

            out_buf = singles.tile([128, JPC, MT], F32, tag="out_buf")

            # --- main loop ------------------------------------------------
            rep_ctx = (tc.For_i(0, repeat, 1,
                                hint_engines=(mybir.EngineType.PE,
                                              mybir.EngineType.DVE,
                                              mybir.EngineType.SP))
                       if repeat > 1 else contextlib.nullcontext())
            with rep_ctx:
                for jg in range(NGROUPS):
                    rt = [[None] * KT for _ in range(JG)]
                    it = [[None] * KT for _ in range(JG)]
                    pt = [[None] * KT for _ in range(JG)]
                    for jj in range(JG):
                        j = jg * JG + jj
                        # one tile per (stream, jj); per-kt views carved below
                        rtile = rpool.tile([128, KT, D], F32R, tag=f"r{jj}",
                                           name=f"r{jg}_{jj}")
                        itile = ipool.tile([128, KT, D], F32R, tag=f"i{jj}",
                                           name=f"i{jg}_{jj}")
                        ptile = ppool.tile([128, KT, D], F32R, tag=f"p{jj}",
                                           name=f"p{jg}_{jj}")
                        for kt in range(KT):
                            rt[jj][kt] = rtile[:, kt, :]
                            it[jj][kt] = itile[:, kt, :]
                            pt[jj][kt] = ptile[:, kt, :]
                        if jg == 0:
                            # split first group's loads so the first matmuls
                            # start as soon as one k-tile has landed
                            for kt in range(KT):
                                ksl = slice(kt * 128, (kt + 1) * 128)
                                nc.sync.dma_start(out=rt[jj][kt],
                                                  in_=r[j, ksl, :])
                                nc.scalar.dma_start(out=it[jj][kt],
                                                    in_=im[j, ksl, :])
                                nc.gpsimd.dma_start(out=pt[jj][kt],
                                                    in_=rpi[j, ksl, :])
                        else:
                            rview = r[j].rearrange("(kt p) b -> p kt b", p=128)
                            iview = im[j].rearrange("(kt p) b -> p kt b", p=128)
                            pview = rpi[j].rearrange("(kt p) b -> p kt b",
                                                     p=128)
                            nc.sync.dma_start(out=rtile, in_=rview)
                            nc.scalar.dma_start(out=itile, in_=iview)
                            nc.gpsimd.dma_start(out=ptile, in_=pview)

                    for m in range(MT):
                        ms = bass.ts(m, 128)
                        ps1 = [psp.tile([128, D], F32, tag=f"p1_{jj}",
                                        name=f"p1_{jg}_{m}_{jj}")
                               for jj in range(JG)]
                        ps2 = [psp.tile([128, D], F32, tag=f"p2_{jj}",
                                        name=f"p2_{jg}_{m}_{jj}")
                               for jj in range(JG)]
                        ps3 = [psp.tile([128, D], F32, tag=f"p3_{jj}",
                                        name=f"p3_{jg}_{m}_{jj}")
                               for jj in range(JG)]
                        for jj in range(JG):
                            for kt in range(KT):
                                first, last = kt == 0, kt == KT - 1
                                nc.tensor.matmul(
                                    ps1[jj][:, :], krT_s[:, kt, ms],
                                    rt[jj][kt], start=first, stop=last)
                                nc.tensor.matmul(
                                    ps2[jj][:, :], kiT_s[:, kt, ms],
                                    it[jj][kt], start=first, stop=last)
                                nc.tensor.matmul(
                                    ps3[jj][:, :], kpkT_s[:, kt, ms],
                                    pt[jj][kt], start=first, stop=last)

                        # out[j, m-tile] = rowsum(m1*d1 + m2*d2 + m3*d3)
                        for jj in range(JG):
                            j = jg * JG + jj
                            scr = scrp.tile([128, 3, D], F32, tag="scr",
                                            name=f"scr{jg}_{m}_{jj}")
                            s2 = scrp.tile([128, D], F32, tag="s2",
                                           name=f"s2_{jg}_{m}_{jj}")
                            s3 = scrp.tile([128, D], F32, tag="s3",
                                           name=f"s3_{jg}_{m}_{jj}")
                            nc.scalar.copy(out=s2[:, :], in_=ps2[jj][:, :])
                            nc.scalar.copy(out=s3[:, :], in_=ps3[jj][:, :])
                            nc.vector.tensor_tensor(
                                out=scr[:, 0, :], in0=ps1[jj][:, :],
                                in1=d1_s[:, m, :], op=MUL)
                            nc.gpsimd.tensor_mul(
                                scr[:, 1, :], s2[:, :], d2_s[:, m, :])
                            nc.gpsimd.tensor_mul(
                                scr[:, 2, :], s3[:, :], d3_s[:, m, :])
                            nc.vector.tensor_reduce(
                                out=out_buf[:, j, m:m + 1], in_=scr[:, :, :],
                                axis=mybir.AxisListType.XY, op=ADD)

            # --- store ----------------------------------------------------
            nc.sync.dma_start(
                out=out.rearrange("j (m p) -> p j m", p=128),
                in_=out_buf[:, :, :],
            )

    nc.finalize()
    return nc


def _get_nc():
    global _cached_nc
    if _cached_nc is None:
        _cached_nc = _build_nc()
    return _cached_nc


def make_in_maps(input_real, input_imag, kernel):
    input_real = np.ascontiguousarray(input_real, dtype=np.float32)
    input_imag = np.ascontiguousarray(input_imag, dtype=np.float32)
    rpi = input_real + input_imag
    kernel = np.asarray(kernel, dtype=np.float32)
    kr = np.ascontiguousarray(kernel[:, :, 0])
    ki = np.ascontiguousarray(kernel[:, :, 1])
    krT = np.ascontiguousarray(kr.T)
    kiT = np.ascontiguousarray(ki.T)
    kpkT = np.ascontiguousarray(krT + kiT)
    d1 = kr - ki
    d2 = -(kr + ki)
    d3 = ki
    in_maps = []
    for c in range(NCORES):
        sl = slice(c * JPC, (c + 1) * JPC)
        in_maps.append({
            "r": input_real[sl],
            "im": input_imag[sl],
            "rpi": rpi[sl],
            "krT": krT,
            "kiT": kiT,
            "kpkT": kpkT,
            "d1": d1,
            "d2": d2,
            "d3": d3,
        })
    return in_maps


def kernel(input_real, input_imag, kernel):
    nc = _get_nc()
    in_maps = make_in_maps(input_real, input_imag, kernel)
    res = run_bass_kernel_spmd(nc, in_maps, core_ids=list(range(NCORES)))
    return np.concatenate(
        [res.results[c]["out"] for c in range(NCORES)], axis=0
    ).astype(np.float32)
